# revision 5
# baseline (speedup 1.0000x reference)
"""GNN (GraphConv -> SAGEConv -> ChebConv) Bass kernel for 8 trn2 NeuronCores.

Architecture (dst-sharded graph parallel):
- Nodes sharded 8 ways by dst range; per-layer activations republished to a
  replicated HBM gather table; SpMM agg(x)[dst] = sum_e w_e x[src] runs as
  dma_gather of 128-edge chunks (custom SWDGE ucode, 4 queues) + PE matmul
  against a DVE-built weighted one-hot S at a register-indexed PSUM column
  offset. Graph normalizations are folded into per-edge weights on host.
- bf16 everywhere off-chip and on the PE operand path (tables, gathers, S,
  weights, activations); fp32 PSUM accumulation; fp32 output. Halves the
  dominant HBM gather/AllGather traffic and FWL-accelerates PE weight loads.
  (Measured rel err vs fp32 reference ~4.5e-3, gate is 2e-2.)
- Layer-1 feature table is host-built (replicated input) -> no AllGather #0.
- Tables for layers 2-4 are split into 4 AG groups, each AllGather'd as soon
  as its superwindows finish, overlapping the collective with remaining SpMM
  work (table rows laid out (group, core, local); one Shared tile per group
  since the scheduler requires a single writer per Shared tensor).
- Edge indices live in SBUF across all 4 SpMMs (loaded once).
"""
import sys
for _p in ("/opt/trn_rl_repo", "/root/.axon_site/_ro/trn_rl_repo"):
    if _p not in sys.path:
        sys.path.insert(0, _p)

import numpy as np
import ml_dtypes

BF = ml_dtypes.bfloat16
P = 128

CFG = dict(
    N=100000, E=1600000, INF=128, HID=128, OUTF=64,
    NCORES=8, SWN=1024, SPAN=64,
)


def _derive(cfg):
    import os
    d = dict(cfg)
    d["V"] = d["N"] // d["NCORES"]
    d["VP"] = ((d["V"] + P - 1) // P) * P
    d["NSW"] = (d["VP"] + d["SWN"] - 1) // d["SWN"]
    # AG groups: table rows laid out (group, core, local). Each group is
    # AllGather'd independently as soon as its superwindows are written,
    # overlapping the collective with the remaining SpMM work.
    agch = int(os.environ.get("GNN_AGCHUNKS", "4"))
    d["AGCH"] = agch
    gsw = {4: [4, 3, 3, 3], 1: [d["NSW"]]}[agch]
    assert sum(gsw) == d["NSW"]
    groups, tb, s = [], 0, 0
    for n in gsw:
        lst = s * d["SWN"]
        lw = min(d["VP"], (s + n) * d["SWN"]) - lst
        groups.append(dict(sw_last=s + n - 1, lstart=lst, lwidth=lw, tbase=tb))
        tb += d["NCORES"] * lw
        s += n
    d["AGROUPS"] = groups
    d["ROWS"] = tb
    d["ROWSA"] = tb
    # pair windows (int16 gather address windows): tile index, in-tile row
    # offset, rows, global row start
    if agch == 1:
        npair = (tb + 32767) // 32768
        pair = ((tb + npair - 1) // npair + P - 1) // P * P
        d["PWIN"] = [(0, p * pair, min(pair, tb - p * pair), p * pair)
                     for p in range(npair)]
    else:
        d["PWIN"] = [(gi, 0, d["NCORES"] * g["lwidth"], g["tbase"])
                     for gi, g in enumerate(groups)]
        assert all(w[2] <= 32768 for w in d["PWIN"])
    d["NPAIR"] = len(d["PWIN"])
    d["PAIR"] = 0  # unused; kept for compat
    return d


def _chunkify(dl_rel, sw_width, span):
    out = []
    i, n = 0, len(dl_rel)
    while i < n:
        d0 = int(dl_rel[i])
        bank = d0 // 512
        bank_end = min((bank + 1) * 512, max(sw_width, d0 + 1))
        d0 = min(d0, max(bank * 512, bank_end - span))
        hi = min(d0 + span, bank_end)
        j = i
        while j < n and j - i < P and dl_rel[j] < hi:
            j += 1
        out.append((i, j, d0))
        i = j
    return out


def preprocess(src, dst, cfg):
    c_ = _derive(cfg)
    NCORES, V, VP, SWN, SPAN, NSW, NPAIR, PAIR = (
        c_["NCORES"], c_["V"], c_["VP"], c_["SWN"], c_["SPAN"], c_["NSW"],
        c_["NPAIR"], c_["PAIR"])
    N = c_["N"]

    src = np.asarray(src).astype(np.int64)
    dst = np.asarray(dst).astype(np.int64)
    deg_in = np.bincount(dst, minlength=N).astype(np.float32)
    deg_out = np.bincount(src, minlength=N).astype(np.float32)
    norm_in = np.maximum(deg_in, 1.0) ** -0.5
    norm_out = np.maximum(deg_out, 1.0) ** -0.5
    inv_deg = 1.0 / np.maximum(deg_in, 1.0)
    nrm = norm_in

    core_of = dst // V
    vloc = src % V
    kcore = src // V
    srow = np.empty(len(src), np.int64)
    for g in c_["AGROUPS"]:
        m = (vloc >= g["lstart"]) & (vloc < g["lstart"] + g["lwidth"])
        srow[m] = g["tbase"] + kcore[m] * g["lwidth"] + (vloc[m] - g["lstart"])
    pstarts = np.asarray([w[3] for w in c_["PWIN"]] + [c_["ROWS"]], np.int64)
    pair_of = np.searchsorted(pstarts, srow, side="right") - 1
    sloc = srow - pstarts[pair_of]

    buckets = [[[None] * NPAIR for _ in range(NSW)] for _ in range(NCORES)]
    chunked = [[[None] * NPAIR for _ in range(NSW)] for _ in range(NCORES)]
    for c in range(NCORES):
        sel = np.nonzero(core_of == c)[0]
        dl = dst[sel] - c * V
        sw_of = dl // SWN
        for swi in range(NSW):
            m1 = sw_of == swi
            sw_width = min(SWN, VP - swi * SWN)
            for p in range(NPAIR):
                m = m1 & (pair_of[sel] == p)
                eids = sel[m]
                order = np.argsort(dl[m], kind="stable")
                eids = eids[order]
                dl_rel = (dst[eids] - c * V - swi * SWN).astype(np.int64)
                buckets[c][swi][p] = (eids, dl_rel)
                chunked[c][swi][p] = _chunkify(dl_rel, sw_width, SPAN)

    NCH = [[0] * NPAIR for _ in range(NSW)]
    for swi in range(NSW):
        for p in range(NPAIR):
            NCH[swi][p] = max(len(chunked[c][swi][p]) for c in range(NCORES))
    CT = sum(sum(r) for r in NCH)

    w_all = np.stack([
        norm_out[src] * norm_in[dst],
        inv_deg[dst],
        -(nrm[src] * nrm[dst]),
        -2.0 * (nrm[src] * nrm[dst]),
    ]).astype(np.float32)

    percore = []
    for c in range(NCORES):
        idx_blocks = []
        dstloc = np.full((CT, P), -1.0, np.float32)
        d0a = np.zeros((CT,), np.int32)
        wa = np.zeros((4, CT, P), np.float32)
        cgl = 0
        for swi in range(NSW):
            for p in range(NPAIR):
                nch = NCH[swi][p]
                if nch == 0:
                    continue
                eids, dl_rel = buckets[c][swi][p]
                chunks = chunked[c][swi][p]
                idx = np.zeros((nch * P,), np.int16)
                for j in range(nch):
                    if j < len(chunks):
                        i0, i1, d0 = chunks[j]
                        k = i1 - i0
                        e = eids[i0:i1]
                        idx[j * P : j * P + k] = sloc[e].astype(np.int16)
                        dstloc[cgl + j, :k] = (dl_rel[i0:i1] - d0).astype(np.float32)
                        d0a[cgl + j] = d0
                        wa[:, cgl + j, :k] = w_all[:, e]
                blk = idx.reshape(-1, 16).T
                idx_blocks.append(np.tile(blk, (8, 1)))
                cgl += nch
        assert cgl == CT
        percore.append(dict(
            idx16=np.concatenate(idx_blocks, axis=1).astype(np.int16),
            dstloc=np.ascontiguousarray(dstloc.T).astype(BF),
            d0=d0a.reshape(1, CT),
            w0=np.ascontiguousarray(wa[0].T).astype(BF),
            w1=np.ascontiguousarray(wa[1].T).astype(BF),
            w2=np.ascontiguousarray(wa[2].T).astype(BF),
            w3=np.ascontiguousarray(wa[3].T).astype(BF),
        ))
    return c_, NCH, CT, percore


def _patch_lane_assignment():
    import concourse.tile_sem_assignment as tsa
    import concourse.bass_isa as bass_isa
    import concourse.mybir as mybir
    if getattr(tsa.TileClockTick, "_gnn_patched", False):
        return
    orig = tsa.TileClockTick._assign_tick

    def patched(self, inst):
        if (isinstance(inst, tsa.DMAInst)
                and not isinstance(inst, bass_isa.UserSyncedRemoteDMADescs)
                and inst.engine == mybir.EngineType.Pool):
            q = int(getattr(inst, "queue_num", 0) or 0)
            cnt = getattr(self, "_gnn_qcnt", None)
            if cnt is None:
                cnt = self._gnn_qcnt = {}
            k = cnt.get(q, 0)
            cnt[q] = k + 1
            self.next_sw_dma_idx = (q + 4 * (k % 2)) % self.swdge_sem_count
        return orig(self, inst)

    tsa.TileClockTick._assign_tick = patched
    tsa.TileClockTick._gnn_patched = True


def build_kernel(c_, NCH, CT):
    import os
    import concourse.bass as bass
    import concourse.bacc as bacc
    import concourse.mybir as mybir
    import concourse.tile as tile
    from concourse import library_config
    from concourse.masks import make_identity
    from concourse.tile_rust import add_dep_helper

    _patch_lane_assignment()

    NCORES, VP, ROWS, SWN, SPAN, NSW, NPAIR, PAIR = (
        c_["NCORES"], c_["VP"], c_["ROWS"], c_["SWN"], c_["SPAN"], c_["NSW"],
        c_["NPAIR"], c_["PAIR"])
    HID, OUTF = c_["HID"], c_["OUTF"]
    f32 = mybir.dt.float32
    bf16 = mybir.dt.bfloat16
    PE = mybir.EngineType.PE
    eq, mul, sub = (mybir.AluOpType.is_equal, mybir.AluOpType.mult,
                    mybir.AluOpType.subtract)
    Relu = mybir.ActivationFunctionType.Relu
    Ident = mybir.ActivationFunctionType.Identity

    nc = bacc.Bacc("TRN2", target_bir_lowering=False, debug=False,
                   num_devices=NCORES, num_swdge_queues=4)

    ROWSA = c_["ROWSA"]
    T0_in = nc.dram_tensor("T0", [ROWSA, 128], bf16, kind="ExternalInput")
    idx_in = nc.dram_tensor("idx16", [P, CT * 8], mybir.dt.int16, kind="ExternalInput")
    dstloc_in = nc.dram_tensor("dstloc", [P, CT], bf16, kind="ExternalInput")
    d0_in = nc.dram_tensor("d0", [1, CT], mybir.dt.int32, kind="ExternalInput")
    w_in = [nc.dram_tensor(f"w{i}", [P, CT], bf16, kind="ExternalInput") for i in range(4)]
    W1_in = nc.dram_tensor("W1", [128, HID], bf16, kind="ExternalInput")
    Ws_in = nc.dram_tensor("W_self", [HID, HID], bf16, kind="ExternalInput")
    Wn_in = nc.dram_tensor("W_neigh", [HID, HID], bf16, kind="ExternalInput")
    Wc_in = nc.dram_tensor("W_cheb3", [128, 3, OUTF], bf16, kind="ExternalInput")
    b1_in = nc.dram_tensor("b1", [HID, 1], f32, kind="ExternalInput")
    b2_in = nc.dram_tensor("b2", [HID, 1], f32, kind="ExternalInput")
    b3_in = nc.dram_tensor("b3", [OUTF, 1], f32, kind="ExternalInput")
    iota_in = nc.dram_tensor("iota", [P, SPAN], bf16, kind="ExternalInput")
    out_dram = nc.dram_tensor("out", [VP, OUTF], f32, kind="ExternalOutput")

    rg = [list(range(NCORES))]

    with tile.TileContext(nc) as tc:
        with (
            tc.tile_pool(name="dram", bufs=1, space="DRAM") as dpool,
            tc.tile_pool(name="const", bufs=1) as cpool,
            tc.tile_pool(name="big", bufs=1) as bigpool,
            tc.tile_pool(name="gp", bufs=int(os.environ.get("GNN_GBUFS", "22"))) as gpool,
            tc.tile_pool(name="sp", bufs=8) as spool,
            tc.tile_pool(name="slice", bufs=2) as slpool,
            tc.tile_pool(name="nmp", bufs=2) as nmpool,
            tc.tile_pool(name="wp", bufs=1) as wpool,
            tc.tile_pool(name="pssw", bufs=2, space="PSUM") as ps_sw,
            tc.tile_pool(name="psd", bufs=2, space="PSUM") as ps_d,
            tc.tile_pool(name="pst", bufs=2, space="PSUM") as ps_t,
        ):
            lib = nc.gpsimd.load_library(library_config.mlp)

            # tables[0] unused (layer 1 gathers straight from T0_in).
            # One Shared tile per AG group (the scheduler requires a single
            # writer instruction per Shared tensor).
            tables = [None] + [
                [dpool.tile([NCORES * g["lwidth"], 128], bf16,
                            addr_space="Shared", name=f"T{i}g{gi}")
                 for gi, g in enumerate(c_["AGROUPS"])]
                for i in (1, 2, 3)]
            bounces = [None] + [dpool.tile([VP, 128], bf16, name=f"bounce{i}")
                                for i in (1, 2, 3)]

            iota = cpool.tile([P, SPAN], bf16)
            nc.sync.dma_start(iota[:], iota_in[:])
            ident = cpool.tile([P, P], bf16)
            make_identity(nc, ident[:])
            z512 = cpool.tile([P, 512], bf16)
            nc.vector.memset(z512[:], 0.0)
            W1sb = cpool.tile([128, HID], bf16); nc.sync.dma_start(W1sb[:], W1_in[:])
            Wssb = cpool.tile([HID, HID], bf16); nc.sync.dma_start(Wssb[:], Ws_in[:])
            Wnsb = cpool.tile([HID, HID], bf16); nc.sync.dma_start(Wnsb[:], Wn_in[:])
            Wcsb = cpool.tile([128, 3, OUTF], bf16); nc.sync.dma_start(Wcsb[:], Wc_in[:])
            b1sb = cpool.tile([HID, 1], f32); nc.sync.dma_start(b1sb[:], b1_in[:])
            b2sb = cpool.tile([HID, 1], f32); nc.sync.dma_start(b2sb[:], b2_in[:])
            b3sb = cpool.tile([OUTF, 1], f32); nc.sync.dma_start(b3sb[:], b3_in[:])
            dstloc = cpool.tile([P, CT], bf16); nc.sync.dma_start(dstloc[:], dstloc_in[:])
            d0t = cpool.tile([1, CT], mybir.dt.int32); nc.sync.dma_start(d0t[:], d0_in[:])
            idx_all = cpool.tile([P, CT * 8], mybir.dt.int16)
            nc.sync.dma_start(idx_all[:], idx_in[:])

            h1T = bigpool.tile([P, VP], bf16)    # L1 out; reused as X1T in L3
            h2T = bigpool.tile([P, VP], bf16)    # L2 out = X0
            X1T = h1T

            _ab = os.environ.get("GNN_ABLATE", "")
            _noag = _ab == "no_ag"
            _gonly = _ab == "gather_only"

            AGROUPS = c_["AGROUPS"]

            def do_ag(i, gi, g):
                if _noag:
                    return
                ls, lw = g["lstart"], g["lwidth"]
                nc.gpsimd.collective_compute(
                    "AllGather", mybir.AluOpType.bypass, replica_groups=rg,
                    ins=[bounces[i][ls : ls + lw, :]],
                    outs=[tables[i][gi][: NCORES * lw, :]])

            def do_ags(i, swi):
                for gi, g in enumerate(AGROUPS):
                    if g["sw_last"] == swi:
                        do_ag(i, gi, g)

            qrot = [0]
            HC = int(os.environ.get("GNN_HC", "4"))
            VB = int(os.environ.get("GNN_VB", "8"))
            _psinit = os.environ.get("GNN_PSINIT", "mm")

            def spmm_sw(l, swi, wbuf, coff):
                ps = ps_sw.tile([P, SWN], f32, tag="sw")
                nbank = (SWN + 511) // 512
                if _psinit == "dve":
                    nc.vector.memset(ps[:], 0.0)
                else:
                    for b in range(nbank):
                        bw = min(512, SWN - b * 512)
                        nc.tensor.matmul(out=ps[:, b * 512 : b * 512 + bw],
                                         lhsT=z512[:, :128], rhs=z512[:, :bw],
                                         start=True, stop=False)
                c = coff
                for p in range(NPAIR):
                    nch = NCH[swi][p]
                    if nch == 0:
                        continue
                    ti, off, nr, gstart = c_["PWIN"][p]
                    if l == 0:
                        src_ap = T0_in[gstart : gstart + nr, :]
                    else:
                        src_ap = tables[l][ti][off : off + nr, :]
                    pieces = [(a, min(a + HC, nch)) for a in range(0, nch, HC)]
                    gtiles = []
                    for (a, b) in pieces:
                        Gp = gpool.tile([P, HC, 128], bf16, tag="g", name="G")
                        if _ab != "no_gather":
                            gi = nc.gpsimd.dma_gather(
                                Gp[:, : b - a, :],
                                src_ap,
                                idx_all[:, (c + a) * 8 : (c + b) * 8],
                                (b - a) * P, (b - a) * P, 128,
                                single_packet=False, queue_num=qrot[0] % 4)
                            qrot[0] += 1
                            add_dep_helper(gi.ins, lib.ins, sync=False,
                                           reason="lib before gather")
                        gtiles.append(Gp)
                    vals = None
                    for j in range(nch):
                        if _ab in ("no_mm", "gather_only"):
                            continue
                        if "static_d0" not in _ab:
                            if j % VB == 0:
                                k = min(VB, nch - j)
                                _, vals = nc.values_load_multi_w_load_instructions(
                                    d0t[0:1, c + j : c + j + k], engines=(PE,),
                                    min_val=0, max_val=SWN - SPAN,
                                    skip_runtime_bounds_check=True)
                        if "no_s" not in _ab:
                            S = spool.tile([P, SPAN], bf16, tag="s")
                            nc.vector.scalar_tensor_tensor(
                                S[:], iota[:], dstloc[:, c + j : c + j + 1],
                                wbuf[:, c + j : c + j + 1].to_broadcast([P, SPAN]),
                                eq, mul)
                            Sap = S[:]
                        else:
                            Sap = iota[:]
                        Gap = gtiles[j // HC][:, j % HC, :]
                        if "static_d0" in _ab:
                            nc.tensor.matmul(out=ps[:, 0:SPAN], lhsT=Gap,
                                             rhs=Sap, start=False, stop=False,
                                             skip_group_check=True)
                        else:
                            nc.tensor.matmul(
                                out=ps[:, bass.ds(vals[j % VB], SPAN)],
                                lhsT=Gap, rhs=Sap,
                                start=False, stop=False, skip_group_check=True)
                    c += nch
                if _psinit != "dve":
                    for b in range(nbank):
                        bw = min(512, SWN - b * 512)
                        nc.tensor.matmul(out=ps[:, b * 512 : b * 512 + bw],
                                         lhsT=z512[:, :128], rhs=z512[:, :bw],
                                         start=False, stop=True)
                return ps, c

            def table_write(hT, s0, wd, bounce):
                if _gonly:
                    return
                n128 = wd // P
                nm = nmpool.tile([P, SWN // P, P], bf16, tag="nm")
                for w8 in range(n128):
                    pt = ps_t.tile([P, P], bf16, tag="t")
                    nc.tensor.transpose(out=pt[:], in_=hT[:, s0 + w8 * P : s0 + (w8 + 1) * P],
                                        identity=ident[:])
                    nc.vector.tensor_copy(nm[:, w8, :], pt[:])
                nc.sync.dma_start(
                    bounce[s0 : s0 + wd, :].rearrange("(w p) f -> p w f", p=P),
                    nm[:, :n128, :])

            layer_w = []
            for l in range(4):
                wb = wpool.tile([P, CT], bf16, tag="w")
                nc.sync.dma_start(wb[:], w_in[l][:])
                layer_w.append(wb)

            REPEAT = int(os.environ.get("GNN_REPEAT", "1"))
            for _rep in range(REPEAT):
              # ---------------- Layer 1 ----------------
              coff = 0
              for swi in range(NSW):
                s0 = swi * SWN
                wd = min(SWN, VP - s0)
                ps, coff = spmm_sw(0, swi, layer_w[0], coff)
                agg = slpool.tile([P, SWN], bf16, tag="swsl")
                nc.vector.tensor_copy(agg[:, :wd], ps[:, :wd])
                for t in range((wd + 511) // 512):
                    w512 = min(512, wd - t * 512)
                    pd = ps_d.tile([P, 512], f32, tag="d")
                    nc.tensor.matmul(out=pd[:, :w512], lhsT=W1sb[:],
                                     rhs=agg[:, t * 512 : t * 512 + w512],
                                     start=True, stop=True)
                    nc.scalar.activation(h1T[:, s0 + t * 512 : s0 + t * 512 + w512],
                                         pd[:, :w512], Relu, bias=b1sb[:])
                table_write(h1T, s0, wd, bounces[1])
            do_ag(1)

            # ---------------- Layer 2 ----------------
            coff = 0
            for swi in range(NSW):
                s0 = swi * SWN
                wd = min(SWN, VP - s0)
                ps, coff = spmm_sw(1, swi, layer_w[1], coff)
                agg = slpool.tile([P, SWN], bf16, tag="swsl")
                nc.vector.tensor_copy(agg[:, :wd], ps[:, :wd])
                for t in range((wd + 511) // 512):
                    w512 = min(512, wd - t * 512)
                    pd = ps_d.tile([P, 512], f32, tag="d")
                    nc.tensor.matmul(out=pd[:, :w512], lhsT=Wssb[:],
                                     rhs=h1T[:, s0 + t * 512 : s0 + t * 512 + w512],
                                     start=True, stop=False)
                    nc.tensor.matmul(out=pd[:, :w512], lhsT=Wnsb[:],
                                     rhs=agg[:, t * 512 : t * 512 + w512],
                                     start=False, stop=True)
                    nc.scalar.activation(h2T[:, s0 + t * 512 : s0 + t * 512 + w512],
                                         pd[:, :w512], Relu, bias=b2sb[:])
                table_write(h2T, s0, wd, bounces[2])
            do_ag(2)

            # ---------------- Layer 3a: X1 = -Ahat(X0) ----------------
            coff = 0
            for swi in range(NSW):
                s0 = swi * SWN
                wd = min(SWN, VP - s0)
                ps, coff = spmm_sw(2, swi, layer_w[2], coff)
                nc.vector.tensor_copy(X1T[:, s0 : s0 + wd], ps[:, :wd])
                table_write(X1T, s0, wd, bounces[3])
            do_ag(3)

            # ------- Layer 3b: X2 = -2 Ahat(X1) - X0; out = Xt @ Wc + b3 -----
            coff = 0
            for swi in range(NSW):
                s0 = swi * SWN
                wd = min(SWN, VP - s0)
                ps, coff = spmm_sw(3, swi, layer_w[3], coff)
                x2 = slpool.tile([P, SWN], bf16, tag="swsl")
                nc.vector.tensor_tensor(x2[:, :wd], ps[:, :wd],
                                        h2T[:, s0 : s0 + wd], sub)
                osl = slpool.tile([OUTF, SWN], bf16, tag="osl")
                for t in range((wd + 511) // 512):
                    w512 = min(512, wd - t * 512)
                    pc = ps_d.tile([OUTF, 512], f32, tag="d")
                    rhss = [h2T[:, s0 + t * 512 : s0 + t * 512 + w512],
                            X1T[:, s0 + t * 512 : s0 + t * 512 + w512],
                            x2[:, t * 512 : t * 512 + w512]]
                    for k in range(3):
                        nc.tensor.matmul(out=pc[:, :w512], lhsT=Wcsb[:, k, :],
                                         rhs=rhss[k], start=(k == 0),
                                         stop=(k == 2))
                    nc.scalar.activation(osl[:, t * 512 : t * 512 + w512],
                                         pc[:, :w512], Ident, bias=b3sb[:])
                n128 = wd // P
                onm = nmpool.tile([P, SWN // P, OUTF], f32, tag="onm")
                for w8 in range(n128):
                    pt = ps_t.tile([P, P], bf16, tag="t")
                    nc.tensor.transpose(out=pt[:, :OUTF],
                                        in_=osl[:, w8 * P : (w8 + 1) * P],
                                        identity=ident[:OUTF, :OUTF])
                    nc.vector.tensor_copy(onm[:, w8, :], pt[:, :OUTF])
                nc.sync.dma_start(
                    out_dram[s0 : s0 + wd, :].rearrange("(w p) f -> p w f", p=P),
                    onm[:, :n128, :])

    nc.compile()
    return nc


def _make_inputs(c_, percore, feat, W1, b1, W_self, W_neigh, b2, W_cheb, b3):
    NCORES, V, VP, SPAN = c_["NCORES"], c_["V"], c_["VP"], c_["SPAN"]
    OUTF, HID = c_["OUTF"], c_["HID"]
    ROWSA = c_["ROWSA"]
    feat = np.asarray(feat, np.float32)
    iota = np.tile(np.arange(SPAN, dtype=np.float32)[None, :], (P, 1)).astype(BF)
    Wc3 = np.ascontiguousarray(
        np.asarray(W_cheb, np.float32).reshape(3, 128, OUTF).transpose(1, 0, 2)
    ).astype(BF)
    T0 = np.zeros((ROWSA, 128), BF)
    for g in c_["AGROUPS"]:
        ls, lw, tb = g["lstart"], g["lwidth"], g["tbase"]
        valid = max(0, min(lw, V - ls))
        for c in range(NCORES):
            T0[tb + c * lw : tb + c * lw + valid] = \
                feat[c * V + ls : c * V + ls + valid].astype(BF)
    in_maps = []
    for c in range(NCORES):
        pc = percore[c]
        in_maps.append(dict(
            T0=T0, idx16=pc["idx16"], dstloc=pc["dstloc"],
            d0=pc["d0"], w0=pc["w0"], w1=pc["w1"], w2=pc["w2"], w3=pc["w3"],
            W1=np.asarray(W1, np.float32).astype(BF),
            W_self=np.asarray(W_self, np.float32).astype(BF),
            W_neigh=np.asarray(W_neigh, np.float32).astype(BF),
            W_cheb3=Wc3,
            b1=np.asarray(b1, np.float32).reshape(HID, 1),
            b2=np.asarray(b2, np.float32).reshape(HID, 1),
            b3=np.asarray(b3, np.float32).reshape(OUTF, 1),
            iota=iota,
        ))
    return in_maps


_CACHE = {}


def kernel(feat, src, dst, W1, b1, W_self, W_neigh, b2, W_cheb, b3):
    from concourse.bass_utils import run_bass_kernel_spmd

    c_, NCH, CT, percore = preprocess(src, dst, CFG)
    key = ("k2", CT, tuple(tuple(r) for r in NCH))
    if key not in _CACHE:
        _CACHE[key] = build_kernel(c_, NCH, CT)
    nc = _CACHE[key]
    in_maps = _make_inputs(c_, percore, feat, W1, b1, W_self, W_neigh, b2,
                           W_cheb, b3)
    res = run_bass_kernel_spmd(nc, in_maps, core_ids=list(range(c_["NCORES"])))
    N, V, OUTF = c_["N"], c_["V"], c_["OUTF"]
    out = np.zeros((N, OUTF), np.float32)
    for c in range(c_["NCORES"]):
        out[c * V : (c + 1) * V] = res.results[c]["out"][:V]
    return out
